# revision 22
# baseline (speedup 1.0000x reference)
"""ALiBi attention (B=4, T=2048, C=672, H=14, D=48) on 8 trn2 NeuronCores.

Key observation: the ALiBi bias max(col-row,0)*slope is exactly zero inside
the causal region (col<=row), so the module reduces to plain causal attention
with scale 1/sqrt(48).

Sharding: core c -> batch b=c//2, head-half s=c%2 (7 of the 14 heads).
Each core computes its heads' attention and a partial output projection
(rows of w_proj for its heads); the host sums the two partials per batch.

Layout strategy per core (all compute in bf16, accum fp32):
  - qkT = [w_q|w_k]^T @ x^T -> [672, 2048] feature-major, repacked per
    head-pair at partition bases {0, 64} (heads A/B of a pair).
  - v in natural [T, 48] layout per head with a ones column appended: the AV
    matmul then yields o rows 0..47 per head and the softmax denominator in
    row 48 (row 112 for head B).
  - attention runs in 512-query chunks (qc). Scores for heads A and B are
    issued back-to-back into adjacent PSUM banks of a 6-bank ring: their
    K=48 contractions live at row groups {0,1} vs {2,3} so the PE streams
    them concurrently. exp on ScalarE is fused over runs of up to 3 ring
    banks (one ACTIVATE per run) to amortize the ~310-cycle ACT overhead --
    ScalarE is the pacing engine for the attention phase.
  - AV for heads A/B writes col groups {0,1}/{2,3} of one accumulating PSUM
    bank (tile_position), also concurrent in the PE.
  - causal handling: spans trimmed to j<=i plus a 16-col spill strip, plus
    [128,256] triangular mask multiplies replicating the reference's
    bf16-quantized index comparison.
  - phases B (qk proj), D (v proj), F (out proj) use the same PSUM ring
    banks as rotating accumulators.
"""

import math
import os
import sys
from contextlib import ExitStack

import numpy as np

if not any(os.path.isdir(os.path.join(p, "concourse")) for p in sys.path):
    sys.path.insert(0, "/opt/trn_rl_repo")

import ml_dtypes  # noqa: E402

import concourse.bass as bass  # noqa: E402
import concourse.mybir as mybir  # noqa: E402
from concourse import tile  # noqa: E402
from concourse.bass_utils import run_bass_kernel_spmd  # noqa: E402

BF16 = ml_dtypes.bfloat16

T = 2048
C = 672
H = 14
D = 48
B = 4
HPC = 7          # heads per core
CH = HPC * D     # 336 per-core head channels
SCALE = 1.0 / math.sqrt(D)
RING = 6         # psum ring banks (each [128, 512] fp32)

# K tiles over the C=672 contraction dim
KT = [(i * 128, min(128, C - i * 128)) for i in range((C + 127) // 128)]


def _install_tile_drain_patch():
    """walrus CoreV3 allows only one sync-wait per ctrl instruction; the
    TileContext exit drain can carry several. Split them across drains."""
    import concourse.tile as _tile
    from concourse.tile import ScopedClock

    if getattr(_tile.TileContext, "_drain_patched", False):
        return

    def _patched(self, tick_clock, wait_clock):
        drain_inst = self.nc.sync.drain()
        wait_clock.add_sem_waits(
            drain_inst.ins, ScopedClock({None: tick_clock.global_clock})
        )
        si = drain_inst.ins.sync_info
        if si is not None and len(si.on_wait) > 1:
            waits = list(si.on_wait)
            si.on_wait = waits[:1]
            drain_inst.ins.sync_info = si
            for i in range(1, len(waits)):
                extra = self.nc.sync.drain()
                extra.ins.sync_info = mybir.SyncInfo(
                    on_wait=waits[i : i + 1], on_update=[]
                )
        self.nc.all_engine_barrier()
        assert self.sems is not None
        popped = self.nc._tile_sem_poison_stack.pop()
        assert popped is self._sem_poison
        self.nc.clear_and_free_semaphores(list(self.sems.allocated().values()))
        self.nc.all_engine_barrier()

    _tile.TileContext._drain_and_barrier = _patched
    _tile.TileContext._drain_patched = True


def _row_pieces(row0, nrows):
    """Split absolute rows [row0, row0+nrows) of the 672-row qkT layout into
    (tile_idx, tile_row_offset, count) pieces along 128-row tiles."""
    pieces = []
    r = row0
    while r < row0 + nrows:
        t = r // 128
        off = r % 128
        cnt = min(128 - off, row0 + nrows - r)
        pieces.append((t, off, cnt))
        r += cnt
    return pieces


def _split_dma_waits(nc):
    """This walrus build accepts only ONE sync-wait per instruction. Hoist
    excess waits onto same-engine NoOps placed just before the instruction."""
    ctr = [0]
    for fn in nc.m.functions:
        for blk in fn.blocks:
            new_list = []
            changed = False
            for inst in blk.instructions:
                si = getattr(inst, "sync_info", None)
                if si is not None and len(si.on_wait) > 1:
                    changed = True
                    waits = list(si.on_wait)
                    for w in waits[:-1]:
                        ctr[0] += 1
                        nop = mybir.InstNoOp(name=f"xw-{ctr[0]}")
                        nop.engine = inst.engine
                        nop.sync_info = mybir.SyncInfo(on_wait=[w], on_update=[])
                        new_list.append(nop)
                    si.on_wait = waits[-1:]
                    inst.sync_info = si
                new_list.append(inst)
            if changed:
                blk.instructions = new_list


def build():
    _install_tile_drain_patch()
    nc = bass.Bass()
    bf = mybir.dt.bfloat16
    f32 = mybir.dt.float32

    xt_ext = nc.declare_dram_parameter("xt", [C, T], bf, isOutput=False)
    wqk_ext = nc.declare_dram_parameter("wqk", [C, 2 * CH], bf, isOutput=False)
    wv_ext = nc.declare_dram_parameter("wv", [C, CH], bf, isOutput=False)
    wp_ext = nc.declare_dram_parameter("wp", [CH, C], bf, isOutput=False)
    # Per-key-tile causal masks replicating the reference's bf16-quantized
    # index comparison (bf16(i) >= bf16(j)): queries can attend up to ~15
    # future positions near the diagonal. Mask kt covers queries
    # [128kt-128, 128kt+128) x keys [128kt, 128kt+128), laid out [128, 16*256].
    mask_ext = nc.declare_dram_parameter("qmask", [128, 16 * 256], bf, isOutput=False)
    out_ext = nc.declare_dram_parameter("out", [T, C], bf, isOutput=True)

    with tile.TileContext(nc) as tc, ExitStack() as ctx:
        # scores psum: [128,1024] tiles (2 banks), head A in cols 0-511 and
        # head B in 512-1023 so one fused ACTIVATE covers both heads. bufs=3
        # so the PE can fill tile i+1 while ACT exps tile i and AV still
        # reads p mirrors of tile i-1 (Tile WAR deps are tile-granular).
        spool = ctx.enter_context(tc.tile_pool(name="spsum", bufs=3, space="PSUM"))
        opool = ctx.enter_context(tc.tile_pool(name="opsum", bufs=2, space="PSUM"))
        pool = ctx.enter_context(tc.tile_pool(name="persist", bufs=1))
        work = ctx.enter_context(tc.tile_pool(name="work", bufs=2))
        ppool = ctx.enter_context(tc.tile_pool(name="pp", bufs=3))
        bcp = ctx.enter_context(tc.tile_pool(name="bcast", bufs=4))

        # ---- load inputs: interleave wqk[i] with xt[i] col-chunk 0 so the
        # first B matmul group can start after just two transfers ------------
        XT, WQK, WV = [], [], []
        for i, (k0, kn) in enumerate(KT):
            XT.append(pool.tile([kn, T], bf, tag=f"xt{i}", name=f"xt{i}"))
            WV.append(pool.tile([kn, CH], bf, tag=f"wv{i}", name=f"wv{i}"))
            WQK.append(pool.tile([kn, 2 * CH], bf, tag=f"wqk{i}", name=f"wqk{i}"))
        # wqk arrives in B's m-tile column order and xt in n-chunk order so
        # the first B chain can start after ~1MB of transfers
        def load_wqk(m):
            m0, mn = KT[m]
            for i, (k0, kn) in enumerate(KT):
                nc.sync.dma_start(
                    WQK[i][:, m0 : m0 + mn], wqk_ext[k0 : k0 + kn, m0 : m0 + mn]
                )

        def load_xt(n0):
            for i, (k0, kn) in enumerate(KT):
                nc.sync.dma_start(
                    XT[i][:, n0 : n0 + 512], xt_ext[k0 : k0 + kn, n0 : n0 + 512]
                )

        load_wqk(0)
        load_xt(0)
        load_xt(512)
        load_wqk(2)
        load_xt(1024)
        load_wqk(3)
        load_xt(1536)
        for m in (1, 4, 5):
            load_wqk(m)
        for i, (k0, kn) in enumerate(KT):
            nc.sync.dma_start(WV[i][:], wv_ext[k0 : k0 + kn, :])
        WP = []
        for i, (k0, kn) in enumerate([(0, 128), (128, 128), (256, 80)]):
            wp_t = pool.tile([kn, C], bf, tag=f"wp{i}")
            nc.sync.dma_start(wp_t[:], wp_ext[k0 : k0 + kn, :])
            WP.append(wp_t)
        qmask = pool.tile([128, 16 * 256], bf, tag="qmask")
        nc.sync.dma_start(qmask[:], mask_ext[:])

        # ---- phase B: qkT = [wq|wk].T @ xT  -> [672, 2048] bf16 ----------
        QKT = [None] * len(KT)
        for m in (0, 2, 3, 1, 4, 5):
            m0, mn = KT[m]
            qk_sb = pool.tile([mn, T], bf, tag=f"qkt{m}", name=f"qkt{m}")
            for n0 in range(0, T, 1024):
                ps = spool.tile([128, 1024], f32, tag="s", name="bps")
                for half in range(2):
                    hs = n0 + half * 512
                    for ki, (k0, kn) in enumerate(KT):
                        nc.tensor.matmul(
                            ps[:mn, half * 512 : half * 512 + 512],
                            WQK[ki][:, m0 : m0 + mn],
                            XT[ki][:, hs : hs + 512],
                            start=(ki == 0),
                            stop=(ki == len(KT) - 1),
                        )
                nc.vector.tensor_copy(qk_sb[:, n0 : n0 + 1024], ps[:mn, :])
            QKT[m] = qk_sb

        # ---- phase C: repack per head-pair at partition bases {0, 64} ----
        # qpair[p] rows 0..48 = q of head 2p; rows 64..112 = q of head 2p+1
        QP, KP = [], []
        for p in range(4):
            qp = pool.tile([128, T], bf, tag=f"qp{p}")
            kp = pool.tile([128, T], bf, tag=f"kp{p}")
            QP.append(qp)
            KP.append(kp)
        for h in range(HPC):
            p, rb = h // 2, 64 * (h % 2)
            for dst, row0 in ((QP[p], h * D), (KP[p], CH + h * D)):
                o = 0
                for (t, off, cnt) in _row_pieces(row0, D):
                    nc.sync.dma_start(
                        dst[rb + o : rb + o + cnt, :], QKT[t][off : off + cnt, :]
                    )
                    o += cnt

        # ---- phase D: v in natural layout + ones column ------------------
        # v_aug layout: [128, HPC, 16, 49]; per (head h, key tile kt) the
        # [128, 49] slice is lhsT for AV (col 48 = ones -> denominator row).
        v_aug = pool.tile([128, HPC * 16 * 49], bf, tag="vaug")
        v4 = v_aug[:].rearrange("p (h t d) -> p h t d", h=HPC, t=16, d=49)
        nc.vector.memset(v4[:, :, :, 48:49], 1.0)
        for t in range(16):
            ps = spool.tile([128, 1024], f32, tag="s", name="dps")
            for ki, (k0, kn) in enumerate(KT):
                nc.tensor.matmul(
                    ps[:, :CH],
                    XT[ki][:, t * 128 : (t + 1) * 128],
                    WV[ki][:],
                    start=(ki == 0),
                    stop=(ki == len(KT) - 1),
                )
            src3 = ps[:, :CH].rearrange("p (h d) -> p h d", h=HPC)
            nc.vector.tensor_copy(v4[:, :, t, 0:48], src3)

        # ---- phase E: attention, per head-pair, 512-query chunks ---------
        # Per (qc, kt): one [128,1024] psum tile holds head A's scores in
        # cols 0-511 and head B's in 512-1023 (their K=48 contractions live
        # at row groups {0,1} vs {2,3} so the PE streams them concurrently);
        # one fused ACTIVATE exps both; AV accumulates both heads into one
        # o_ps bank via col groups. Pair 3 (lone head) packs two consecutive
        # key tiles per psum tile instead.
        OPK = [pool.tile([128, T], bf, tag=f"opk{i}", name=f"opk{i}") for i in range(3)]
        PKT = [(0, 128), (128, 128), (256, 80)]

        def emit_F(qc):
            # y = opk.T @ wp for this query chunk's four token tiles; runs
            # interleaved with pair 3's attention (which has PE slack)
            for t in range(4 * qc, 4 * qc + 4):
                ysb = work.tile([128, C], bf, tag="ysb")
                fps = spool.tile([128, 1024], f32, tag="s", name=f"fps{t}")
                for n0, nn in ((0, 512), (512, 160)):
                    for ki, (k0, kn) in enumerate(PKT):
                        nc.tensor.matmul(
                            fps[:, n0 : n0 + nn],
                            OPK[ki][:kn, t * 128 : (t + 1) * 128],
                            WP[ki][:, n0 : n0 + nn],
                            start=(ki == 0),
                            stop=(ki == 2),
                        )
                nc.vector.tensor_copy(ysb[:], fps[:, :C])
                nc.sync.dma_start(out_ext[t * 128 : (t + 1) * 128, :], ysb[:])

        for p in range(4):
            heads = [(2 * p, 0)] if p == 3 else [(2 * p, 0), (2 * p + 1, 64)]
            yh = work.tile([128, T], bf, tag="yh", name=f"yh{p}")
            for qc in range(4):
                # F for chunk qc-1 lands here, one chunk behind pair 3's
                # attention, so its epilogue/OPK dependencies are long done
                # by the time the PE FIFO reaches it
                if p == 3 and qc > 0:
                    emit_F(qc - 1)
                o_ps = opool.tile([128, 512], f32, tag="o", name=f"o{p}_{qc}")
                kt_last = min(4 * qc + 4, 15)
                # groups of up to 2 items (h, rb, kt, ls0, col_half) per tile
                groups = []
                if len(heads) == 2:
                    for kt in range(kt_last + 1):
                        ls0 = max(0, kt * 128 - 16 - qc * 512)
                        groups.append(
                            [
                                (heads[0][0], heads[0][1], kt, ls0, 0),
                                (heads[1][0], heads[1][1], kt, ls0, 1),
                            ]
                        )
                else:
                    h6, rb6 = heads[0]
                    cur = []
                    for kt in range(kt_last + 1):
                        ls0 = max(0, kt * 128 - 16 - qc * 512)
                        if cur and (cur[0][3] != ls0 or len(cur) == 2):
                            groups.append(cur)
                            cur = []
                        cur.append((h6, rb6, kt, ls0, len(cur)))
                    if cur:
                        groups.append(cur)
                # batch-2 emission: two groups of scores stream before their
                # AVs so consecutive same-row-group matmuls hide array drains
                pend = []
                for gi, g in enumerate(groups):
                    ps = spool.tile([128, 1024], f32, tag="s", name=f"s{p}_{qc}_{gi}")
                    pt = ppool.tile([128, 1024], bf, tag="p", name=f"p{p}_{qc}_{gi}")
                    ls0 = g[0][3]
                    n = len(g)
                    for (h, rb, kt, l0, ch) in g:
                        nc.tensor.matmul(
                            ps[:, ch * 512 + l0 : ch * 512 + 512],
                            KP[p][rb : rb + D, kt * 128 : (kt + 1) * 128],
                            QP[p][rb : rb + D, qc * 512 + l0 : (qc + 1) * 512],
                            start=True,
                            stop=True,
                        )
                    # one contiguous exp for the whole tile; for trimmed
                    # (diagonal) tiles the gap cols [512:512+ls0] hold stale
                    # psum that gets exp'd harmlessly (never read by AV)
                    nc.scalar.activation(
                        pt[:, ls0 : n * 512],
                        ps[:, ls0 : n * 512],
                        mybir.ActivationFunctionType.Exp,
                        scale=SCALE,
                    )
                    pend.append((g, pt))
                    if len(pend) < 2 and gi != len(groups) - 1:
                        continue
                    for (g2, pt2) in pend:
                        for (h, rb, kt, l0, ch) in g2:
                            # masked diagonal region of this (qc, kt); on
                            # GpSimd so the exp->mask->AV chain never queues
                            # behind the DVE's reciprocal / divide work
                            mg0 = kt * 128 - 128
                            dls = max(l0, mg0 - qc * 512)
                            dle = min(512, kt * 128 + 128 - qc * 512)
                            if dls < dle:
                                mo = 256 * kt + (qc * 512 + dls - mg0)
                                nc.gpsimd.tensor_mul(
                                    pt2[:, ch * 512 + dls : ch * 512 + dle],
                                    pt2[:, ch * 512 + dls : ch * 512 + dle],
                                    qmask[:, mo : mo + (dle - dls)],
                                )
                            nc.tensor.matmul(
                                o_ps[rb : rb + D + 1, l0:512],
                                v4[:, h, kt, :],
                                pt2[:, ch * 512 + l0 : (ch + 1) * 512],
                                start=(kt == 0),
                                stop=(kt == kt_last),
                                tile_position=(0, rb),
                                skip_group_check=True,
                            )
                    pend = []

                # epilogue: softmax divide for this query chunk. First move
                # o_ps to SBUF with a fast ScalarE copy so the PSUM bank
                # frees immediately (next qc's AV reuses it via opool);
                # everything downstream runs off the SBUF copy. Denominator
                # sits in rows 48 (head A) / 112 (head B); reciprocal of the
                # full tile is free-size-bound so numerator rows just
                # produce unused garbage.
                o_sb = bcp.tile([128, 512], f32, tag="osb")
                nc.scalar.copy(o_sb[:, :], o_ps[:, :])
                den = bcp.tile([128, 512], f32, tag="den")
                nc.vector.reciprocal(den[:, :], o_sb[:, :])
                for (h, rb) in heads:
                    # log-doubling partition broadcast of the reciprocal row
                    bc = bcp.tile([128, 512], f32, tag="bc")
                    nc.sync.dma_start(bc[rb : rb + 1, :], den[rb + D : rb + D + 1, :])
                    filled = 1
                    while filled < D:
                        nn = min(filled, D - filled)
                        nc.sync.dma_start(
                            bc[rb + filled : rb + filled + nn, :], bc[rb : rb + nn, :]
                        )
                        filled += nn
                    nc.vector.tensor_mul(
                        yh[rb : rb + D, qc * 512 : (qc + 1) * 512],
                        o_sb[rb : rb + D, :],
                        bc[rb : rb + D, :],
                    )
                # repack this query chunk into OPK; once pair 3 (processed
                # last) finishes a chunk, all heads are present and the
                # output projection for its token tiles can run
                cols = slice(qc * 512, (qc + 1) * 512)
                for (h, rb) in heads:
                    o = 0
                    for (t, off, cnt) in _row_pieces(h * D, D):
                        nc.sync.dma_start(
                            OPK[t][off : off + cnt, cols],
                            yh[rb + o : rb + o + cnt, cols],
                        )
                        o += cnt
                if p == 3 and qc == 3:
                    emit_F(3)

    _split_dma_waits(nc)
    return nc


_NC_CACHE = None


def _get_nc():
    global _NC_CACHE
    if _NC_CACHE is None:
        _NC_CACHE = build()
    return _NC_CACHE


def make_in_maps(x, w_attn, w_proj):
    # bf16-quantized causal masks, one [128, 256] block per key tile kt:
    # mask[j - 128kt, i - (128kt - 128)] = bf16(i) >= bf16(j)
    idx = np.arange(T, dtype=np.float32).astype(BF16).astype(np.float32)
    qm = np.zeros((128, 16 * 256), dtype=np.float32)
    for kt in range(16):
        jg = idx[kt * 128 : (kt + 1) * 128]
        i0 = kt * 128 - 128
        ig = np.where(
            (np.arange(i0, i0 + 256) >= 0) & (np.arange(i0, i0 + 256) < T),
            idx[np.clip(np.arange(i0, i0 + 256), 0, T - 1)],
            -1.0,
        )
        qm[:, kt * 256 : (kt + 1) * 256] = (ig[None, :] >= jg[:, None]).astype(
            np.float32
        )
    qmask = qm.astype(BF16)
    in_maps = []
    for c in range(8):
        b, s = c // 2, c % 2
        xt = np.ascontiguousarray(x[b].T).astype(BF16)
        wq = w_attn[:, s * CH : (s + 1) * CH]
        wk = w_attn[:, C + s * CH : C + (s + 1) * CH]
        wv = w_attn[:, 2 * C + s * CH : 2 * C + (s + 1) * CH]
        wqk = np.concatenate([wq, wk], axis=1).astype(BF16)
        wp = w_proj[s * CH : (s + 1) * CH, :].astype(BF16)
        in_maps.append(
            {
                "xt": xt,
                "wqk": np.ascontiguousarray(wqk),
                "wv": np.ascontiguousarray(wv.astype(BF16)),
                "wp": np.ascontiguousarray(wp),
                "qmask": qmask,
            }
        )
    return in_maps


def run(x, w_attn, w_proj, trace=False):
    nc = _get_nc()
    in_maps = make_in_maps(x, w_attn, w_proj)
    res = run_bass_kernel_spmd(nc, in_maps, core_ids=list(range(8)), trace=trace)
    parts = [res.results[c]["out"].astype(np.float32) for c in range(8)]
    y = np.stack([parts[2 * b] + parts[2 * b + 1] for b in range(B)], axis=0)
    return y.astype(BF16), res


def kernel(x, w_attn, w_proj):
    y, _ = run(np.asarray(x, dtype=np.float32),
               np.asarray(w_attn, dtype=np.float32),
               np.asarray(w_proj, dtype=np.float32))
    return y
